# revision 27
# baseline (speedup 1.0000x reference)
"""Differential attention Trainium2 kernel (Bass/Tile), 8-core SPMD.

reference:
  attn1 = softmax(causal(Q1 K1^T / sqrt(D))) V
  attn2 = softmax(causal(Q2 K2^T / sqrt(D))) V
  out   = attn1 - exp(lambda_log) * attn2
shapes: [B=2, H=12, S=2048, D=128] fp32.

Sharding: B*H = 24 head-batches, 3 per NeuronCore (data/head parallel, no
cross-core comms). Host pre-transposes Q/K to [D, S] layout so the device
needs no on-chip transposes; device returns output d-major ([D, S] per
head) and the host transposes back.

All matmul operands fp16 (PE streams 2-byte dtypes at 1 col/cycle); PSUM
accumulation fp32. Engine budget per core (model): PE 3 streams x 104448
cols = 130.5us; ScalarE exp 104448 cols @1.2GHz + (N+352)/1.2 per-inst
overhead ~ 122us; DVE ~55us. So exp is batched: full k-tiles in PAIRS
through [128,1024] PSUM score supertiles (one ACT per pair), diagonal
tiles as two shrunk chunks (896 + 384 cols, one ACT each).

Engine queues are strict FIFO, so the emission order software-pipelines
consumers one round behind producers to avoid head-of-line blocking:
  per (head, group, pass) [passes SEQUENTIAL so the score supertile gets
  2 buffers from the freed PSUM banks]:
    round jp: QK pair -> st slot; ACT(prev slot done) -> E; then emit the
      PREVIOUS round's sums/PV matmuls (their ACT finished a round ago).
    diag produces (QK + pre-exp band mask via strided copy_predicated +
    ACT) at the end of the pass; its 8 consumer matmuls are DEFERRED into
    the next (group,pass) stream, emitted right after its first QK+ACT.
  finals per group emitted at the pass-1 diag-consumer flush: 2x
  reciprocal_approx_fast (1 DVE op, ~51 ULP), 2 muls, 1 add -- pass-1's
  ones matrix is pre-scaled by -1/lambda host-side so sums1 absorbs the
  -lambda and the last op is a plain add.

PSUM (8 banks): score supertiles [128,1024] x2 bufs (4) + outp x2 bufs
(2) + sums x2 bufs (2); outp/sums rotate (g,pass)-wise. Warmup: 10 dummy
matmuls + 1 dummy exp run during the initial DMA wait so the PE HAM
clock-gate (4/8 cold -> 8/8 after ~3.4us busy) and the ACT exp table are
warm before real work; head-0 loads are sliced so pass-0 group-0 can
start after ~400KB of DMA.
"""

import sys

sys.path.insert(0, "/opt/trn_rl_repo")

import numpy as np

B, H, S, D = 2, 12, 2048, 128
NCORES = 8
BH = B * H
HEADS = BH // NCORES  # 3 heads per core
P = 128
NT = S // P           # 16 key tiles
GW = 512              # query-group width (matmul free dim)
G = S // GW           # 4 query groups
TPG = GW // P         # 4 tiles per group
SCALE = float(D) ** -0.5

_PROGRAM = None


def _build_program():
    import concourse.mybir as mybir
    import concourse.tile as tile
    from concourse import bacc

    fp32 = mybir.dt.float32
    fp16 = mybir.dt.float16
    u8 = mybir.dt.uint8
    Exp = mybir.ActivationFunctionType.Exp

    nc = bacc.Bacc(None)
    qt1 = nc.dram_tensor("qt1", [HEADS, P, S], fp16, kind="ExternalInput")
    kt1 = nc.dram_tensor("kt1", [HEADS, P, S], fp16, kind="ExternalInput")
    qt2 = nc.dram_tensor("qt2", [HEADS, P, S], fp16, kind="ExternalInput")
    kt2 = nc.dram_tensor("kt2", [HEADS, P, S], fp16, kind="ExternalInput")
    vd = nc.dram_tensor("v", [HEADS, P, NT, D], fp16, kind="ExternalInput")
    onesd = nc.dram_tensor("ones", [P, 2, P], fp16, kind="ExternalInput")
    tri = nc.dram_tensor("tri", [P, P], u8, kind="ExternalInput")
    out = nc.dram_tensor("out", [HEADS, P, S], fp32, kind="ExternalOutput")

    with tile.TileContext(nc) as tc:
        with (
            tc.tile_pool(name="const", bufs=1) as cpool,
            tc.tile_pool(name="load", bufs=2) as lpool,
            tc.tile_pool(name="et", bufs=5) as epool,
            tc.tile_pool(name="fin", bufs=2) as fpool,
            tc.tile_pool(name="spsum", bufs=2, space="PSUM") as spool,
            tc.tile_pool(name="opsum", bufs=1, space="PSUM") as opool,
            tc.tile_pool(name="supsum", bufs=1, space="PSUM") as upool,
        ):
            zeros16 = cpool.tile([P, P], fp16)
            nc.vector.memset(zeros16[:], 0.0)
            warm = cpool.tile([P, GW], fp16)
            nc.vector.memset(warm[:], 0.001)

            # warmup: dummy matmuls + one exp run during the initial DMA
            # wait so the PE HAM gate and ACT exp table are warm
            wst = spool.tile([P, 3, GW], fp32, tag="st", name="warm_st")
            for w in range(12):
                nc.tensor.matmul(
                    wst[:, w % 3, :], warm[:, 0:P], warm[:],
                    start=True, stop=True,
                )
            wet = epool.tile([P, 3, GW], fp16, tag="et", name="warm_et")
            nc.scalar.activation(wet[:, 0, :], wst[:, 0, :], Exp, scale=SCALE)

            # ---- load schedule -------------------------------------------
            loaded = {}

            def load_head(h):
                if h in loaded:
                    return loaded[h]
                ts = []
                for name, t in (("q1", qt1), ("k1", kt1),
                                ("q2", qt2), ("k2", kt2)):
                    ts.append(lpool.tile([P, S], fp16, tag=name,
                                         name=f"{name}_{h}"))
                v_s = lpool.tile([P, NT, D], fp16, tag="v", name=f"v_{h}")
                # slice order: everything pass-0 group-0 needs first, then
                # pass-1 group-0, then the rest
                nc.sync.dma_start(ts[0][:, 0:GW], qt1[h][:, 0:GW])
                nc.sync.dma_start(ts[1][:, 0:GW], kt1[h][:, 0:GW])
                nc.sync.dma_start(v_s[:, 0:TPG, :], vd[h][:, 0:TPG, :])
                nc.sync.dma_start(ts[2][:, 0:GW], qt2[h][:, 0:GW])
                nc.sync.dma_start(ts[3][:, 0:GW], kt2[h][:, 0:GW])
                nc.sync.dma_start(ts[1][:, GW:2 * GW], kt1[h][:, GW:2 * GW])
                nc.sync.dma_start(ts[0][:, GW:2 * GW], qt1[h][:, GW:2 * GW])
                nc.sync.dma_start(v_s[:, TPG:2 * TPG, :],
                                  vd[h][:, TPG:2 * TPG, :])
                nc.sync.dma_start(ts[3][:, GW:2 * GW], kt2[h][:, GW:2 * GW])
                nc.sync.dma_start(ts[2][:, GW:2 * GW], qt2[h][:, GW:2 * GW])
                for t_, src in ((ts[1], kt1), (ts[0], qt1),
                                (ts[3], kt2), (ts[2], qt2)):
                    nc.sync.dma_start(t_[:, 2 * GW:], src[h][:, 2 * GW:])
                nc.sync.dma_start(v_s[:, 2 * TPG:, :], vd[h][:, 2 * TPG:, :])
                loaded[h] = (ts, v_s)
                return loaded[h]

            # ---- deferred work ------------------------------------------
            # pending: emitted right after the first QK+ACT of the next
            # (group, pass) stream (list of callables)
            pending = []

            def flush_pending():
                while pending:
                    pending.pop(0)()

            def emit_half_final(h, g, pi, outp, sums, holder):
                # normalize one pass: t_pi = outp_pi / sums_pi (2 DVE ops);
                # pass-0's half runs during pass 1's stream
                def go():
                    rcp = fpool.tile([P, GW], fp32, tag=f"rcp{pi}",
                                     name=f"rcp{pi}_{h}_{g}")
                    nc.vector.reciprocal_approx_fast(rcp[:], sums[:])
                    t_ = fpool.tile([P, GW], fp32, tag=f"t{pi}",
                                    name=f"t{pi}_{h}_{g}")
                    nc.vector.tensor_mul(t_[:], outp[:], rcp[:])
                    holder[pi] = t_
                return go

            def emit_final_sum(h, g, holder):
                def go():
                    fin = fpool.tile([P, GW], fp32, tag="fin",
                                     name=f"fin_{h}_{g}")
                    if h == HEADS - 1 and g == G - 1:
                        # last group: halve so the out-DMA pipelines with
                        # the add instead of extending the tail
                        hw = GW // 2
                        for c0 in (0, hw):
                            nc.vector.tensor_add(
                                fin[:, c0:c0 + hw],
                                holder[0][:, c0:c0 + hw],
                                holder[1][:, c0:c0 + hw],
                            )
                            nc.sync.dma_start(
                                out[h][:, g * GW + c0:g * GW + c0 + hw],
                                fin[:, c0:c0 + hw],
                            )
                    else:
                        nc.vector.tensor_add(fin[:], holder[0][:],
                                             holder[1][:])
                        nc.sync.dma_start(out[h][:, g * GW:(g + 1) * GW],
                                          fin[:])
                return go

            # per-(h,g): outp/sums tiles for both passes (created lazily at
            # pass 0, finals emitted after pass 1's diag consumers)
            acc = {}

            tri_s = cpool.tile([P, P], u8)
            ones_mat = cpool.tile([P, 2, P], fp16)

            for h in range(HEADS):
                qk, v_s = load_head(h)
                if h == 0:
                    # consts after the first compute-critical load slices
                    nc.sync.dma_start(tri_s[:], tri[:])
                    nc.sync.dma_start(ones_mat[:], onesd[:])
                for g in range(G):
                    jfull = TPG * g
                    q0 = g * GW
                    accg = [None, None]
                    tholder = [None, None]
                    acc[(h, g)] = accg
                    for pi in range(2):
                        qcol = qk[2 * pi][:, q0:q0 + GW]
                        ks = qk[2 * pi + 1]
                        one = ones_mat[:, pi, :]
                        outp = opool.tile([P, GW], fp32, tag="outp",
                                          name=f"outp{pi}_{h}_{g}")
                        sums = upool.tile([P, GW], fp32, tag="sums",
                                          name=f"sums{pi}_{h}_{g}")
                        accg[pi] = (outp, sums)

                        feS = [False]
                        feO = [False]

                        def cons_sums(e_ap, col0_, stop_,
                                      sums_=sums, one_=one, fe=feS):
                            start_ = not fe[0]
                            fe[0] = True
                            nc.tensor.matmul(
                                sums_[:, col0_:], one_, e_ap,
                                start=start_, stop=stop_,
                            )

                        def cons_outp(e_ap, j_, col0_, stop_,
                                      outp_=outp, v__=v_s, fe=feO):
                            start_ = not fe[0]
                            fe[0] = True
                            nc.tensor.matmul(
                                outp_[:, col0_:], v__[:, j_, :], e_ap,
                                start=start_, stop=stop_,
                            )

                        # ---- full tiles in rounds of <=3 through the
                        # [128, 3*512] score slots; consumers lag one
                        # round; the ones-matmul streams the DVE-summed
                        # round total once ----
                        prev_cons = None
                        rounds = []
                        j = 0
                        while j < jfull:
                            nt = min(3, jfull - j)
                            rounds.append((j, nt))
                            j += nt
                        for ri, (j0, nt) in enumerate(rounds):
                            st = spool.tile([P, 3, GW], fp32, tag="st",
                                            name=f"st_{h}_{g}_{pi}_{ri}")
                            for t_ in range(nt):
                                nc.tensor.matmul(
                                    st[:, t_, :],
                                    ks[:, (j0 + t_) * P:(j0 + t_ + 1) * P],
                                    qcol, start=True, stop=True,
                                )
                            et = epool.tile([P, 3, GW], fp16, tag="et",
                                            name=f"et_{h}_{g}_{pi}_{ri}")
                            ef = et[:].rearrange("p t q -> p (t q)")
                            nc.scalar.activation(
                                ef[:, 0:nt * GW],
                                st[:].rearrange("p t q -> p (t q)")[
                                    :, 0:nt * GW],
                                Exp, scale=SCALE,
                            )
                            if nt == 1:
                                es = et[:, 0, :]
                            else:
                                est = epool.tile([P, GW], fp16, tag="es",
                                                 name=f"es_{h}_{g}_{pi}_{ri}")
                                nc.vector.tensor_add(est[:], et[:, 0, :],
                                                     et[:, 1, :])
                                es = est[:]
                                if nt == 3:
                                    es3 = epool.tile(
                                        [P, GW], fp16, tag="es3",
                                        name=f"es3_{h}_{g}_{pi}_{ri}")
                                    nc.vector.tensor_add(es3[:], est[:],
                                                         et[:, 2, :])
                                    es = es3[:]
                            if ri == 0:
                                flush_pending()
                            if prev_cons is not None:
                                prev_cons()

                            def mk_cons(et_=et, es_=es, j0_=j0, nt_=nt,
                                        cs_=cons_sums, co_=cons_outp):
                                cs_(es_, 0, False)
                                for t_ in range(nt_):
                                    co_(et_[:, t_, :], j0_ + t_, 0, False)
                            prev_cons = mk_cons

                        # ---- diagonal: all 4 shrunk tiles in one slot,
                        # ONE exp (N=1280), two strided band-zero copies --
                        # each dr's matmul output must stay inside one
                        # 512-col PSUM bank: dr0 fills bank0, dr1 (384) +
                        # dr3 (128) fill bank1, dr2 (256) starts bank2 ->
                        # contiguous [0:1280], no junk columns
                        offs = [0, 512, 896, 1024]
                        offs = [offs[0], offs[1], offs[3], offs[2]]
                        widths = [512, 384, 256, 128]
                        st = spool.tile([P, 3, GW], fp32, tag="st",
                                        name=f"std_{h}_{g}_{pi}")
                        sf = st[:].rearrange("p t q -> p (t q)")
                        for dr in range(4):
                            nc.tensor.matmul(
                                sf[:, offs[dr]:offs[dr] + widths[dr]],
                                ks[:, (jfull + dr) * P:(jfull + dr + 1) * P],
                                qk[2 * pi][:, q0 + dr * P:q0 + GW],
                                start=True, stop=True,
                            )
                        et = epool.tile([P, 3, GW], fp16, tag="et",
                                        name=f"etd_{h}_{g}_{pi}")
                        ef = et[:].rearrange("p t q -> p (t q)")
                        nc.scalar.activation(
                            ef[:, 0:1280], sf[:, 0:1280], Exp, scale=SCALE,
                        )
                        # bands: dr0@0/dr1@512 (stride 512), dr3@896/
                        # dr2@1024 (stride 128)
                        for boff, bstride in ((0, 512), (896, 128)):
                            bands = ef[:, boff:boff + 2 * bstride].rearrange(
                                "p (t c) -> p t c", t=2, c=bstride
                            )[:, :, 0:P]
                            nc.vector.copy_predicated(
                                bands,
                                tri_s[:].rearrange("p c -> p () c")
                                .broadcast_to([P, 2, P]),
                                zeros16[:].rearrange("p c -> p () c")
                                .broadcast_to([P, 2, P]),
                            )
                        if jfull == 0:
                            flush_pending()

                        def mk_diag_cons(ef_=ef, jf_=jfull,
                                         cs_=cons_sums, co_=cons_outp,
                                         offs_=tuple(offs),
                                         widths_=tuple(widths)):
                            for dr in range(4):
                                last_ = dr == 3
                                sl = ef_[:, offs_[dr]:offs_[dr] + widths_[dr]]
                                cs_(sl, dr * P, last_)
                                co_(sl, jf_ + dr, dr * P, last_)
                        diag_cons = [mk_diag_cons]

                        # last full-pair consumers emitted now; diag
                        # consumers + this pass's normalization deferred
                        # into the next stream
                        if prev_cons is not None:
                            prev_cons()
                        pending.extend(d for d in diag_cons)
                        pending.append(
                            emit_half_final(h, g, pi, outp, sums, tholder)
                        )
                        if pi == 1:
                            pending.append(emit_final_sum(h, g, tholder))
                # prefetch next head while this one computes
                if h + 1 < HEADS:
                    load_head(h + 1)
            flush_pending()

    nc.compile()
    return nc


def _get_program():
    global _PROGRAM
    if _PROGRAM is None:
        _PROGRAM = _build_program()
    return _PROGRAM


def _make_in_maps(q1, k1, v, q2, k2, lambda_log):
    lam_val = float(np.exp(np.float64(lambda_log.reshape(-1)[0])))
    ones_np = np.empty((P, 2, P), dtype=np.float16)
    ones_np[:, 0, :] = 1.0
    ones_np[:, 1, :] = -1.0 / lam_val
    # kill-mask for the diagonal band: 1 where k > q (strictly below diag)
    tri_np = (np.arange(P)[:, None] > np.arange(P)[None, :]).astype(np.uint8)

    def t(x):  # [BH, S, D] -> [BH, D, S] contiguous
        return np.ascontiguousarray(
            x.reshape(BH, S, D).transpose(0, 2, 1)
        ).astype(np.float16)

    q1t = t(q1)
    q2t = t(q2)
    k1t = t(k1)
    k2t = t(k2)
    # pre-tile V to [BH, p, j, d] so the SBUF load is contiguous per
    # partition: v_s[p, j, d] = V[128 j + p, d]
    vf = np.ascontiguousarray(
        v.reshape(BH, NT, P, D).transpose(0, 2, 1, 3)
    ).astype(np.float16)

    in_maps = []
    for c in range(NCORES):
        sl = slice(c * HEADS, (c + 1) * HEADS)
        in_maps.append(
            {
                "qt1": q1t[sl],
                "kt1": k1t[sl],
                "qt2": q2t[sl],
                "kt2": k2t[sl],
                "v": vf[sl],
                "ones": ones_np,
                "tri": tri_np,
            }
        )
    return in_maps


def _run(q1, k1, v, q2, k2, lambda_log, trace=False):
    from concourse.bass_utils import run_bass_kernel_spmd

    nc = _get_program()
    in_maps = _make_in_maps(q1, k1, v, q2, k2, lambda_log)
    res = run_bass_kernel_spmd(
        nc, in_maps, core_ids=list(range(NCORES)), trace=trace
    )
    parts = [res.results[c]["out"].transpose(0, 2, 1) for c in range(NCORES)]
    full = np.concatenate(parts, axis=0).reshape(B, H, S, D)
    return np.ascontiguousarray(full, dtype=np.float32), res


def kernel(q1, k1, v, q2, k2, lambda_log):
    out, _ = _run(q1, k1, v, q2, k2, lambda_log, trace=False)
    return out
